# revision 34
# baseline (speedup 1.0000x reference)
"""MoE block (AdaptFormer adapters, top-2 of 8 experts) on 8 TRN2 NeuronCores.

Data-parallel over the 8192 tokens (1024/core), router + expert adapter
weights replicated. Per core, one fused streaming pipeline (no phase split):

  - x ships as an exact bf16 hi/lo split, pre-transposed on the host to
    [D, T]; DMA is a fully serialized priority chain (xh block 0 first)
    so early compute is never starved by concurrent transfers.
  - logits per 512-token block: two accumulation groups -> lt_ps [40, 512]:
    rows 0:16 = xh @ [wgh|wgl] (one 16-col pass), rows 32:40 = xl @ wgh.
    Error ~3e-6, far below the 3.6e-5 min top-2/3 logit gap.
  - gating is batched per block: 4 PE transposes land the 40 logit rows in
    one [128, 4, 40] psum tile; the top-2 softmax (x0.5 adapter scale
    folded in) runs as ~16 DVE/ACT ops on [128, 4, 8] strided views
    (broadcast_to for the per-token max/denominator), then 4 PE
    transposes produce g2T [8, 512] bf16.
  - experts run densely in bf16 (error budget 2e-2; bf16 lands ~4e-3):
    HT chunks = Wd^T x -> relu (bf16), GB = Eblk^T @ g2T expands gates
    across the 512-wide expert axis, hg = relu * GB (bf16), out tiles =
    hg @ Wu accumulated over the expert axis, stored bf16, widened on
    the host.
All experts computed densely; sparse gates zero the non-top-2 terms
(mathematically identical to dispatch/combine).
"""
import numpy as np
import ml_dtypes
from contextlib import ExitStack

import concourse.bass as bass
import concourse.tile as tile
from concourse.tile import add_dep_helper
from concourse import bacc, mybir
from concourse.bass_utils import run_bass_kernel_spmd

N_CORES = 8
B_DIM, S_DIM, D = 2, 4096, 1024
T = B_DIM * S_DIM          # 8192 tokens
TC = T // N_CORES          # 1024 tokens per core
E, BK = 8, 64              # experts, bottleneck
EB = E * BK                # 512 concatenated expert axis
P = 128
KC = D // P                # D chunks
HC = KC // 2               # half of the D chunks (split xh block-0 DMA)
BC = EB // P               # bottleneck chunks
LBLK = 512                 # token block
NLB = TC // LBLK           # 2 blocks per core
TPB = LBLK // P            # token tiles per block
SCALE = 0.5
N_WARM = 6                 # PE warm-up matmuls during initial DMA wait
N_FILL = 4                 # PE fillers while wd streams in
G = 2 * E                  # 16 = width of the combined [wgh|wgl] pass
XB = 32                    # xl-pass rows base (matmul out partition 0/32/64)
LW = XB + E                # 40 = logit psum rows (combined + xl pass)

F32 = mybir.dt.float32
F32R = mybir.dt.float32r
BF16 = mybir.dt.bfloat16
AL = mybir.AluOpType
ACTF = mybir.ActivationFunctionType
AX = mybir.AxisListType

_BUILD_CACHE = {}


def _build(include_bd: bool, include_bu: bool, reps: int = 1):
    key = (include_bd, include_bu, reps)
    if key in _BUILD_CACHE:
        return _BUILD_CACHE[key]

    nc = bacc.Bacc("TRN2", target_bir_lowering=False, debug=False,
                   num_devices=N_CORES)
    # All big inputs ship partition-major (packed on the host) so every
    # DMA is one contiguous slab with 4-8KB per-partition runs — 1KB-row
    # gathers are descriptor-bound at ~200 GB/s.
    xh_d = nc.dram_tensor("xh", [NLB, 2, P, HC, LBLK], BF16,
                          kind="ExternalInput").ap()
    xl_d = nc.dram_tensor("xl", [NLB, 2, P, HC, LBLK], BF16,
                          kind="ExternalInput").ap()
    # wd/wu split by k-halves: an HT k-group consumes every c chunk, so
    # splitting by c would stall it on the second half anyway
    wd_d = nc.dram_tensor("wd", [2, P, KC, EB // 2], BF16,
                          kind="ExternalInput").ap()
    wu_d = nc.dram_tensor("wu", [2, P, 2, D], BF16,
                          kind="ExternalInput").ap()
    # [wgh | wgl] side by side, packed [P, KC, 16] bf16
    wg_d = nc.dram_tensor("wghl", [P, KC, G], BF16,
                          kind="ExternalInput").ap()
    id_d = nc.dram_tensor("ident", [P, P], F32, kind="ExternalInput").ap()
    idb_d = nc.dram_tensor("identb", [P, P], BF16, kind="ExternalInput").ap()
    eb_d = nc.dram_tensor("eblk", [E, EB], BF16, kind="ExternalInput").ap()
    if include_bd:
        bd_d = nc.dram_tensor("bd", [P, BC], F32, kind="ExternalInput").ap()
    if include_bu:
        bu_d = nc.dram_tensor("bu", [E, D], BF16, kind="ExternalInput").ap()
    out_d = nc.dram_tensor("out", [TC, D], BF16, kind="ExternalOutput").ap()

    with tile.TileContext(nc) as tc, ExitStack() as ctx:
        wpool = ctx.enter_context(tc.tile_pool(name="weights", bufs=1))
        hgpool = ctx.enter_context(tc.tile_pool(name="hg", bufs=8))
        rpool = ctx.enter_context(tc.tile_pool(name="relu", bufs=8))
        gpool = ctx.enter_context(tc.tile_pool(name="gates", bufs=2))
        opool = ctx.enter_context(tc.tile_pool(name="osb", bufs=3))

        htgb_ps_pool = ctx.enter_context(
            tc.tile_pool(name="htgb", bufs=2, space="PSUM"))
        lt_ps_pool = ctx.enter_context(
            tc.tile_pool(name="ltps", bufs=1, space="PSUM"))
        small_ps_pool = ctx.enter_context(
            tc.tile_pool(name="smps", bufs=1, space="PSUM"))
        g2t_ps_pool = ctx.enter_context(
            tc.tile_pool(name="g2tps", bufs=1, space="PSUM"))
        o_ps_pool = ctx.enter_context(
            tc.tile_pool(name="ops", bufs=3, space="PSUM"))

        # PE warm-up first: source tile is memset (no DMA wait); keeps the
        # HAM clock un-throttled while the first block's inputs stream in.
        warm32 = wpool.tile([P, LBLK], F32, tag="warm32")
        nc.vector.memset(warm32[:], 0.001)
        warm_src = wpool.tile([P, LBLK], BF16, tag="warmsrc")
        nc.vector.tensor_copy(warm_src[:], warm32[:])
        warm_ps = o_ps_pool.tile([P, LBLK], F32, tag="ops")

        def emit_warm(n):
            for _ in range(n):
                nc.tensor.matmul(warm_ps[:], warm_src[:, 0:P], warm_src[:],
                                 start=True, stop=True)

        emit_warm(N_WARM)

        # ---- priority DMA ladder on the sync queue: uniform ~512KB
        # transfers, each waiting for the one three back (~3 in flight
        # hides the ~1.2us handoff; full serialization costs ~2us per
        # transfer, free-for-all loses priority) ----
        hist = []

        def dma(dst, src):
            i = nc.sync.dma_start(dst, src)
            if len(hist) >= 4:
                add_dep_helper(i.ins, hist[-4].ins, sync=True,
                               reason="dma priority ladder")
            hist.append(i)
            return i

        # tiny constants go on the gpsimd DGE queue, off the ladder
        wg_sb = wpool.tile([P, KC, G], BF16, tag="wghl")
        nc.gpsimd.dma_start(wg_sb[:], wg_d)
        ident = wpool.tile([P, P], F32, tag="ident")
        nc.gpsimd.dma_start(ident[:], id_d)
        ident_b = wpool.tile([P, P], BF16, tag="identb")
        nc.gpsimd.dma_start(ident_b[:], idb_d)
        eblk = wpool.tile([E, EB], BF16, tag="eblk")
        nc.gpsimd.dma_start(eblk[:], eb_d)
        if include_bd:
            bd_sb = wpool.tile([P, BC], F32, tag="bd")
            nc.gpsimd.dma_start(bd_sb[:], bd_d)
        if include_bu:
            bu_sb = wpool.tile([E, D], BF16, tag="bu")
            nc.gpsimd.dma_start(bu_sb[:], bu_d)

        # xh/xl per (block, half-of-D), wd/wu per k-half: ~512KB tiles so
        # compute starts on the first slab and streams behind the ladder
        xh_sb = [[wpool.tile([P, HC, LBLK], BF16, tag=f"xh{b}{h}",
                             name=f"xh{b}{h}") for h in range(2)]
                 for b in range(NLB)]
        xl_sb = [[wpool.tile([P, HC, LBLK], BF16, tag=f"xl{b}{h}",
                             name=f"xl{b}{h}") for h in range(2)]
                 for b in range(NLB)]
        wd_sb = [wpool.tile([P, KC, EB // 2], BF16, tag=f"wd{h}",
                            name=f"wd{h}") for h in range(2)]
        wu_sb = [wpool.tile([P, 2, D], BF16, tag=f"wu{h}",
                            name=f"wu{h}") for h in range(2)]

        def xh_c(b, c):
            return xh_sb[b][c // HC][:, c % HC, :]

        def xl_c(b, c):
            return xl_sb[b][c // HC][:, c % HC, :]

        def wd_ck(c, k):
            return wd_sb[k // 2][:, c, bass.ts(k % 2, P)]

        def wu_kh(k, h):
            return wu_sb[k // 2][:, k % 2, bass.ts(h, 512)]

        dma(xh_sb[0][0][:], xh_d[0, 0])
        dma(xh_sb[0][1][:], xh_d[0, 1])
        dma(wd_sb[0][:], wd_d[0])
        dma(xl_sb[0][0][:], xl_d[0, 0])
        dma(xl_sb[0][1][:], xl_d[0, 1])
        dma(wd_sb[1][:], wd_d[1])
        dma(xh_sb[1][0][:], xh_d[1, 0])
        dma(xh_sb[1][1][:], xh_d[1, 1])
        dma(xl_sb[1][0][:], xl_d[1, 0])
        dma(xl_sb[1][1][:], xl_d[1, 1])
        dma(wu_sb[0][:], wu_d[0])
        dma(wu_sb[1][:], wu_d[1])

        def emit_logits_c(blk):
            """Combined [wgh|wgl] pass -> lt_ps rows 0:16."""
            lt_ps = lt_ps_pool.tile([LW, LBLK], F32, tag="ltps",
                                    name=f"lt{blk}")
            for c in range(KC):
                nc.tensor.matmul(lt_ps[0:G, :], wg_sb[:, c, :], xh_c(blk, c),
                                 start=(c == 0), stop=(c == KC - 1))
            return lt_ps

        def emit_logits_xl(blk, lt_ps):
            """xl @ wgh pass -> lt_ps rows 32:40, then copy to SBUF."""
            for c in range(KC):
                nc.tensor.matmul(lt_ps[XB:LW, :], wg_sb[:, c, 0:E],
                                 xl_c(blk, c),
                                 start=(c == 0), stop=(c == KC - 1))
            lt_sb = gpool.tile([LW, LBLK], F32, tag="ltsb")
            nc.scalar.copy(lt_sb[:], lt_ps[:])
            return lt_sb

        def emit_ltT(lt_sb):
            """4 transposes: logit rows for the whole block into PSUM."""
            small = small_ps_pool.tile([P, TPB, LW + 8], F32, tag="smps")
            for t in range(TPB):
                nc.tensor.transpose(small[:, t, 0:LW],
                                    lt_sb[:, bass.ts(t, P)],
                                    ident[0:LW, 0:LW])
            return small

        def emit_chain(small, blk):
            """Batched top-2 softmax (x0.5) for all 512 tokens of a block."""
            l24 = gpool.tile([P, TPB, LW], F32, tag="l24")
            nc.scalar.copy(l24[:], small[:, :, 0:LW])
            l_s = gpool.tile([P, TPB, E], F32, tag="lpart")
            nc.vector.tensor_tensor(l_s[:], l24[:, :, 0:E], l24[:, :, E:G],
                                    op=AL.add)
            l_sb = gpool.tile([P, TPB, E], F32, tag="lsb")
            nc.vector.tensor_tensor(l_sb[:], l_s[:], l24[:, :, XB:LW],
                                    op=AL.add)
            sh3 = [P, TPB, E]
            m1 = gpool.tile([P, TPB, 1], F32, tag="m1")
            nc.vector.tensor_reduce(m1[:, :, 0], l_sb[:], AX.X, AL.max)
            mask1 = gpool.tile(sh3, F32, tag="mask1")
            nc.vector.tensor_tensor(mask1[:], l_sb[:],
                                    m1[:].broadcast_to(sh3), op=AL.is_ge)
            lm = gpool.tile(sh3, F32, tag="lm")
            nc.vector.scalar_tensor_tensor(
                lm[:], mask1[:], -1e30, l_sb[:], op0=AL.mult, op1=AL.add)
            m2 = gpool.tile([P, TPB, 1], F32, tag="m2")
            nc.vector.tensor_reduce(m2[:, :, 0], lm[:], AX.X, AL.max)
            e2m = gpool.tile([P, TPB, 1], F32, tag="e2m")
            nc.vector.tensor_tensor(e2m[:], m2[:], m1[:], op=AL.subtract)
            e2 = gpool.tile([P, TPB, 1], F32, tag="e2")
            nc.scalar.activation(e2[:], e2m[:], ACTF.Exp)
            d2 = gpool.tile([P, TPB, 1], F32, tag="d2")
            nc.scalar.activation(d2[:], e2[:], ACTF.Copy,
                                 bias=1.0 / SCALE, scale=1.0 / SCALE)
            rh = gpool.tile([P, TPB, 1], F32, tag="rh")
            nc.vector.reciprocal(rh[:], d2[:])
            lsh = gpool.tile(sh3, F32, tag="lsh")
            nc.vector.tensor_tensor(lsh[:], l_sb[:],
                                    m1[:].broadcast_to(sh3), op=AL.subtract)
            expl = gpool.tile(sh3, F32, tag="expl")
            nc.scalar.activation(expl[:], lsh[:], ACTF.Exp)
            mask2 = gpool.tile(sh3, F32, tag="mask2")
            nc.vector.tensor_tensor(mask2[:], l_sb[:],
                                    m2[:].broadcast_to(sh3), op=AL.is_ge)
            t1 = gpool.tile(sh3, F32, tag="t1")
            nc.vector.tensor_tensor(t1[:], expl[:], mask2[:], op=AL.mult)
            g2 = gpool.tile(sh3, BF16, tag="g2", name=f"g2_{blk}")
            nc.vector.tensor_tensor(g2[:], t1[:],
                                    rh[:].broadcast_to(sh3), op=AL.mult)
            return g2

        def emit_g2T(g2):
            """4 transposes: gates back to [8, tok] bf16 in SBUF."""
            g2t_ps = g2t_ps_pool.tile([E, LBLK], BF16, tag="g2tps")
            for t in range(TPB):
                nc.tensor.transpose(g2t_ps[:, bass.ts(t, P)], g2[:, t, :],
                                    ident_b[:])
            g2t_sb = gpool.tile([E, LBLK], BF16, tag="g2t")
            nc.scalar.copy(g2t_sb[:], g2t_ps[:])
            return g2t_sb

        def emit_ht(blk, k):
            """HT chunk k: relu(Wd^T x) in bf16."""
            ht_ps = htgb_ps_pool.tile([P, LBLK], F32, tag="htps")
            for c in range(KC):
                nc.tensor.matmul(ht_ps[:], wd_ck(c, k), xh_c(blk, c),
                                 start=(c == 0), stop=(c == KC - 1))
            r_k = rpool.tile([P, LBLK], BF16, tag="relu")
            if include_bd:
                nc.scalar.activation(r_k[:], ht_ps[:], ACTF.Relu,
                                     bias=bd_sb[:, k:k + 1])
            else:
                nc.scalar.activation(r_k[:], ht_ps[:], ACTF.Relu)
            return r_k

        def emit_gb(k, g2t_sb):
            """Gate-expand matmul for chunk k."""
            gb_ps = htgb_ps_pool.tile([P, LBLK], F32, tag="htps")
            nc.tensor.matmul(gb_ps[:], eblk[:, bass.ts(k, P)], g2t_sb[:],
                             start=True, stop=True)
            return gb_ps

        def emit_hg(blk, k, r_k, gb_ps):
            """hg = relu * gates (bf16, DVE)."""
            hg_k = hgpool.tile([P, LBLK], BF16, tag="hg",
                               name=f"hg{blk}_{k}")
            nc.vector.tensor_tensor(hg_k[:], r_k[:], gb_ps[:], op=AL.mult)
            return hg_k

        def emit_out(blk, hgs, g2t_sb, last=False):
            """out tiles = HG @ Wu (+ g2 @ bu); one 256KB store per tile
            (2KB per-partition rows keep the store DMA off the
            descriptor-bound path). The final tile stores its halves
            separately to shorten the kernel tail."""
            for bo in range(TPB):
                t = blk * TPB + bo
                rows = bass.ts(t, P)
                tok = bass.ts(bo, P)
                split = last and bo == TPB - 1
                o_sb = opool.tile([P, D], BF16, tag="osb")
                for h in range(2):
                    o_ps = o_ps_pool.tile([P, 512], F32, tag="ops")
                    for k in range(BC):
                        nc.tensor.matmul(
                            o_ps[:], hgs[k][:, tok], wu_kh(k, h),
                            start=(k == 0),
                            stop=(k == BC - 1 and not include_bu))
                    if include_bu:
                        nc.tensor.matmul(o_ps[:], g2t_sb[:, tok],
                                         bu_sb[:, bass.ts(h, 512)],
                                         start=False, stop=True)
                    if h == 0:
                        nc.vector.tensor_copy(o_sb[:, 0:512], o_ps[:])
                    else:
                        nc.scalar.copy(o_sb[:, 512:D], o_ps[:])
                    if split:
                        nc.sync.dma_start(out_d[rows, bass.ts(h, 512)],
                                          o_sb[:, bass.ts(h, 512)])
                # stores ride the sync queue: it is idle once the load
                # ladder drains, and store issues on the ACT queue would
                # delay the block-1 gating copies behind them
                if not split:
                    nc.sync.dma_start(out_d[rows, :], o_sb[:])

        for rep in range(reps):
            # ---- block 0: logits stream behind the xh DMA; the gating
            # chain (DVE/ACT) hides under HT matmuls ----
            lt0 = emit_logits_c(0)
            emit_warm(N_FILL)
            r0 = [emit_ht(0, k) for k in range(2)]
            lt_sb0 = emit_logits_xl(0, lt0)
            small0 = emit_ltT(lt_sb0)
            g2_0 = emit_chain(small0, 0)
            r0 += [emit_ht(0, k) for k in range(2, BC)]
            g2t0 = emit_g2T(g2_0)
            gbs0 = [emit_gb(k, g2t0) for k in range(BC)]
            hgs0 = [emit_hg(0, k, r0[k], gbs0[k]) for k in range(BC)]

            # ---- block 1 gating; its chain hides under HT1/OUT0.
            # chain1 + g2T1 are emitted before OUT0 so their ACT ops are
            # not queued behind OUT0's copies ----
            lt1 = emit_logits_c(1)
            r1 = [emit_ht(1, 0)]
            lt_sb1 = emit_logits_xl(1, lt1)
            small1 = emit_ltT(lt_sb1)
            g2_1 = emit_chain(small1, 1)
            r1 += [emit_ht(1, k) for k in range(1, BC)]
            g2t1 = emit_g2T(g2_1)
            emit_out(0, hgs0, g2t0)
            gbs1 = [emit_gb(k, g2t1) for k in range(BC)]
            hgs1 = [emit_hg(1, k, r1[k], gbs1[k]) for k in range(BC)]
            emit_out(1, hgs1, g2t1, last=(rep == reps - 1))

    nc.compile()
    _BUILD_CACHE[key] = nc
    return nc


def _split_bf16(a):
    hi = a.astype(ml_dtypes.bfloat16)
    lo = (a - hi.astype(np.float32)).astype(ml_dtypes.bfloat16)
    return hi, lo


def kernel(x, w_gate, w_noise, Wd, bd, Wu, bu, reps: int = 1):
    x = np.ascontiguousarray(np.asarray(x, dtype=np.float32))
    assert x.shape == (B_DIM, S_DIM, D), x.shape
    wg = np.ascontiguousarray(np.asarray(w_gate, dtype=np.float32))
    Wd = np.asarray(Wd, dtype=np.float32)
    Wu = np.asarray(Wu, dtype=np.float32)
    bd = np.asarray(bd, dtype=np.float32)
    bu = np.asarray(bu, dtype=np.float32)

    include_bd = bool(np.any(bd))
    include_bu = bool(np.any(bu))
    nc = _build(include_bd, include_bu, reps)

    xf = x.reshape(T, D)
    xh, xl = _split_bf16(xf)
    xht_full = np.ascontiguousarray(xh.T)   # [D, T]
    xlt_full = np.ascontiguousarray(xl.T)
    wgh, wgl = _split_bf16(wg)
    wghl = np.concatenate([wgh, wgl], axis=1)          # [D, 16] bf16
    # partition-major packs: (c*128+p, n) -> [p, c, n]
    wghl_p = np.ascontiguousarray(
        wghl.reshape(KC, P, G).transpose(1, 0, 2))
    wd_all = Wd.transpose(1, 0, 2).reshape(D, EB).astype(ml_dtypes.bfloat16)
    # [2, P, KC, EB//2]: k-halves outermost
    wd_p = np.ascontiguousarray(
        wd_all.reshape(KC, P, 2, EB // 2).transpose(2, 1, 0, 3))
    wu_flat = Wu.reshape(EB, D).astype(ml_dtypes.bfloat16)
    # [2, P, 2, D]: (k//2, p, k%2, d)
    wu_p = np.ascontiguousarray(
        wu_flat.reshape(2, 2, P, D).transpose(0, 2, 1, 3))
    ident = np.eye(P, dtype=np.float32)
    eblk = np.kron(np.eye(E, dtype=np.float32),
                   np.ones((1, BK), dtype=np.float32))  # [E, EB]

    shared = dict(wd=wd_p, wu=wu_p, wghl=wghl_p, ident=ident,
                  identb=ident.astype(ml_dtypes.bfloat16),
                  eblk=eblk.astype(ml_dtypes.bfloat16))
    if include_bd:
        # [P, BC] partition-major per chunk: bd_sb[p, k] = bd_flat[128k+p]
        shared["bd"] = np.ascontiguousarray(
            bd.reshape(EB)[np.arange(P)[:, None] + P * np.arange(BC)[None]])
    if include_bu:
        shared["bu"] = np.ascontiguousarray(bu).astype(ml_dtypes.bfloat16)

    def pack_xh(xt):
        # [D, TC] -> [NLB, 2, P, HC, LBLK]: (h*HC+c2)*P+p, b*LBLK+t
        a = xt.reshape(2, HC, P, NLB, LBLK)
        return np.ascontiguousarray(a.transpose(3, 0, 2, 1, 4))

    in_maps = []
    for c in range(N_CORES):
        sl = slice(c * TC, (c + 1) * TC)
        in_maps.append(dict(xh=pack_xh(xht_full[:, sl]),
                            xl=pack_xh(xlt_full[:, sl]),
                            **shared))
    kernel.last_in_maps = in_maps
    res = run_bass_kernel_spmd(nc, in_maps, core_ids=list(range(N_CORES)))
    out = np.concatenate([res.results[c]["out"].astype(np.float32)
                          for c in range(N_CORES)], axis=0)
    return out.reshape(B_DIM, S_DIM, D)
